# revision 3
# baseline (speedup 1.0000x reference)
"""TRN2 Bass kernel for nn_CoreAttention_34875134444341.

Strategy (8 NeuronCores, no collectives):
  - Data-parallel over batch (4) x causal-balanced query-row split (2).
  - Each core: full K/V projections for its batch; Q projection for its
    1024 query rows (zig-zag tile assignment balances causal attention
    work); block attention in "transposed" orientation (keys on the
    partition axis) so no on-chip transposes are needed; final Wo matmul
    row-parallel (no reduction across cores).
  - All matmuls run as float32r (full-rate fp32 on the PE array) except
    the Wo stage which runs bf16.
  - Host gathers per-core outputs and inverse-permutes rows.
"""

import sys

sys.path.insert(0, "/opt/trn_rl_repo")

import numpy as np
import ml_dtypes

B, S, D = 4, 2048, 2048
H, HKV, DK = 16, 4, 128
RQ = RKV = 512
GROUP = H // HKV
P = 128

TILE_R = 256  # query rows per slot
KB = 128  # keys per block
NB_SCHED = [16, 12, 8, 4]  # key blocks per slot (same on every core)
TILE_ASSIGN = {0: [7, 5, 2, 0], 1: [6, 4, 3, 1]}  # slot -> query tile

ROWS_PER_CORE = 4 * TILE_R  # 1024

_CACHE = {}
TRACE = False
LAST_RESULT = None


def _rows_sched(parity):
    return np.concatenate(
        [np.arange(t * TILE_R, (t + 1) * TILE_R) for t in TILE_ASSIGN[parity]]
    )


def _make_mask(parity):
    """[128 key_in_block, 4 slots, 2 pairs, 512 (=2 blocks x 256 rows)] additive."""
    m = np.zeros((KB, 4, 2, 2 * TILE_R), np.float32)
    for s in range(4):
        t = TILE_ASSIGN[parity][s]
        nb = NB_SCHED[s]
        row_g = t * TILE_R + np.arange(TILE_R)
        for pr in range(2):  # last two pairs of the slot's schedule
            for half in range(2):
                blk = nb - 4 + 2 * pr + half
                key_g = blk * KB + np.arange(KB)
                bad = key_g[:, None] > row_g[None, :]
                sl = m[:, s, pr, half * TILE_R : (half + 1) * TILE_R]
                sl[bad] = -1e30
    return m


def _build_nc():
    import concourse.tile as tile
    from concourse import bacc, mybir

    f32 = mybir.dt.float32
    f32r = mybir.dt.float32r
    bf16 = mybir.dt.bfloat16
    Exp = mybir.ActivationFunctionType.Exp
    Mult = mybir.AluOpType.mult
    Add = mybir.AluOpType.add

    nc = bacc.Bacc("TRN2", target_bir_lowering=False, debug=False)

    xTkv = nc.dram_tensor("xTkv", [D, S], f32r, kind="ExternalInput")
    xTq = nc.dram_tensor("xTq", [D, ROWS_PER_CORE], f32r, kind="ExternalInput")
    wq1 = nc.dram_tensor("wq1", [D, RQ], f32r, kind="ExternalInput")
    wq2 = nc.dram_tensor("wq2", [RQ, H * DK], f32r, kind="ExternalInput")
    wk1 = nc.dram_tensor("wk1", [D, RKV], f32r, kind="ExternalInput")
    wk2 = nc.dram_tensor("wk2", [RKV, HKV * DK], f32r, kind="ExternalInput")
    wv1 = nc.dram_tensor("wv1", [D, RKV], f32r, kind="ExternalInput")
    wv2 = nc.dram_tensor("wv2", [RKV, HKV * DK], f32r, kind="ExternalInput")
    wo = nc.dram_tensor("wo", [D, D], bf16, kind="ExternalInput")
    maskin = nc.dram_tensor("maskin", [KB, 4, 2, 2 * TILE_R], f32, kind="ExternalInput")
    ones_in = nc.dram_tensor("ones_in", [P, 1], f32r, kind="ExternalInput")
    out = nc.dram_tensor("out", [ROWS_PER_CORE, D], f32, kind="ExternalOutput")

    # DRAM scratch (internal)
    kT_dram = nc.dram_tensor("kT_scratch", [HKV, P, S], f32r)
    v_dram = nc.dram_tensor("v_scratch", [S // P, P, HKV * DK], f32r)
    qT_dram = nc.dram_tensor("qT_scratch", [H, P, ROWS_PER_CORE], f32r)

    xTkv_t = xTkv.rearrange("(dc p) s -> p dc s", p=P)  # [128, 16, 2048]
    xTq_t = xTq.rearrange("(dc p) r -> p dc r", p=P)  # [128, 16, 1024]
    wq1_t = wq1.rearrange("(dc p) r -> p dc r", p=P)  # [128, 16, 512]
    wk1_t = wk1.rearrange("(dc p) r -> p dc r", p=P)
    wv1_t = wv1.rearrange("(dc p) r -> p dc r", p=P)
    wq2_t = wq2.rearrange("(rc p) h -> p rc h", p=P)  # [128, 4, 2048]
    wk2_t = wk2.rearrange("(rc p) h -> p rc h", p=P)  # [128, 4, 512]
    wv2_t = wv2.rearrange("(rc p) h -> p rc h", p=P)
    wo_t = wo.rearrange("(hc p) o -> p hc o", p=P)  # [128, 16, 2048]

    with tile.TileContext(nc) as tc:
        with tc.tile_pool(name="persist", bufs=1) as persist:
            ones_sb = persist.tile([P, 1], f32r)
            nc.sync.dma_start(ones_sb[:], ones_in[:])
            mask_sb = persist.tile([P, 4, 2, 2 * TILE_R], f32)
            nc.sync.dma_start(mask_sb[:], maskin[:])

            # ------- Phase 1: K then V projections -> DRAM scratch --------
            with (
                tc.tile_pool(name="kv_w1", bufs=1) as kv_w1,
                tc.tile_pool(name="kv_w2", bufs=1) as kv_w2,
                tc.tile_pool(name="kv_x", bufs=2) as kv_x,
                tc.tile_pool(name="kv_mid", bufs=2) as kv_mid,
                tc.tile_pool(name="kv_out", bufs=3) as kv_out,
                tc.tile_pool(name="kv_ps", bufs=4, space="PSUM") as kv_ps,
            ):
                for which in range(2):  # 0 = K, 1 = V
                    w1_t, w2_t = (wk1_t, wk2_t) if which == 0 else (wv1_t, wv2_t)
                    w1_sb = kv_w1.tile([P, 16, RKV], f32r, tag="w1")
                    w2_sb = kv_w2.tile([P, 4, HKV * DK], f32r, tag="w2")
                    nc.sync.dma_start(w1_sb[:], w1_t)
                    nc.sync.dma_start(w2_sb[:], w2_t)

                    for tcn in range(4):  # token 512-chunks
                        xt = kv_x.tile([P, 16, 512], f32r, tag="xt")
                        nc.sync.dma_start(
                            xt[:], xTkv_t[:, :, tcn * 512 : (tcn + 1) * 512]
                        )
                        mid = kv_mid.tile([P, 4, 512], f32r, tag="mid")
                        for rc in range(4):
                            ps_1 = kv_ps.tile([P, 512], f32, tag="ps1")
                            for dc in range(16):
                                nc.tensor.matmul(
                                    ps_1[:],
                                    w1_sb[:, dc, rc * P : (rc + 1) * P],
                                    xt[:, dc],
                                    start=(dc == 0),
                                    stop=(dc == 15),
                                )
                            nc.any.tensor_copy(mid[:, rc], ps_1[:])

                        if which == 0:
                            # kT [kvh*128, tokens]
                            for hc in range(HKV):
                                ps_2 = kv_ps.tile([P, 512], f32, tag="ps2")
                                for rc in range(4):
                                    nc.tensor.matmul(
                                        ps_2[:],
                                        w2_sb[:, rc, hc * P : (hc + 1) * P],
                                        mid[:, rc],
                                        start=(rc == 0),
                                        stop=(rc == 3),
                                    )
                                bounce = kv_out.tile([P, 512], f32r, tag="bo")
                                nc.any.tensor_copy(bounce[:], ps_2[:])
                                nc.sync.dma_start(
                                    kT_dram[hc, :, tcn * 512 : (tcn + 1) * 512],
                                    bounce[:],
                                )
                        else:
                            # v [tokens, kvhid]
                            for i in range(4):
                                ps_2 = kv_ps.tile([P, 512], f32, tag="ps2")
                                for rc in range(4):
                                    nc.tensor.matmul(
                                        ps_2[:],
                                        mid[:, rc, i * P : (i + 1) * P],
                                        w2_sb[:, rc],
                                        start=(rc == 0),
                                        stop=(rc == 3),
                                    )
                                bounce = kv_out.tile([P, 512], f32r, tag="bo")
                                nc.any.tensor_copy(bounce[:], ps_2[:])
                                nc.sync.dma_start(
                                    v_dram[tcn * 4 + i, :, :], bounce[:]
                                )

            # ------- Phase 2: Q projection -> DRAM scratch ----------------
            with (
                tc.tile_pool(name="q_w", bufs=1) as q_w,
                tc.tile_pool(name="q_x", bufs=2) as q_x,
                tc.tile_pool(name="q_mid", bufs=1) as q_mid,
                tc.tile_pool(name="q_out", bufs=3) as q_out,
                tc.tile_pool(name="q_ps", bufs=4, space="PSUM") as q_ps,
            ):
                wq1_sb = q_w.tile([P, 16, RQ], f32r)
                wq2_sb = q_w.tile([P, 4, H * DK], f32r)
                nc.sync.dma_start(wq1_sb[:], wq1_t)
                nc.sync.dma_start(wq2_sb[:], wq2_t)

                q1t = q_mid.tile([P, 4, ROWS_PER_CORE], f32r)
                for tcn in range(2):
                    xt = q_x.tile([P, 16, 512], f32r, tag="xtq")
                    nc.sync.dma_start(xt[:], xTq_t[:, :, tcn * 512 : (tcn + 1) * 512])
                    for rc in range(4):
                        ps_q = q_ps.tile([P, 512], f32, tag="psq1")
                        for dc in range(16):
                            nc.tensor.matmul(
                                ps_q[:],
                                wq1_sb[:, dc, rc * P : (rc + 1) * P],
                                xt[:, dc],
                                start=(dc == 0),
                                stop=(dc == 15),
                            )
                        nc.any.tensor_copy(
                            q1t[:, rc, tcn * 512 : (tcn + 1) * 512], ps_q[:]
                        )
                for h in range(H):
                    for tcn in range(2):
                        ps_qT = q_ps.tile([P, 512], f32, tag="psq2")
                        for rc in range(4):
                            nc.tensor.matmul(
                                ps_qT[:],
                                wq2_sb[:, rc, h * P : (h + 1) * P],
                                q1t[:, rc, tcn * 512 : (tcn + 1) * 512],
                                start=(rc == 0),
                                stop=(rc == 3),
                            )
                        qbounce = q_out.tile([P, 512], f32r, tag="qb")
                        nc.any.tensor_copy(qbounce[:], ps_qT[:])
                        nc.sync.dma_start(
                            qT_dram[h, :, tcn * 512 : (tcn + 1) * 512], qbounce[:]
                        )

            # ------- Phases 3+4 share attn_all ----------------------------
            with tc.tile_pool(name="attn_keep", bufs=1) as attn_keep:
                attn_all = attn_keep.tile([P, 4, H, TILE_R], bf16)

                # ---- Phase 3: attention ----
                with (
                    tc.tile_pool(name="at_kv", bufs=1) as at_kv,
                    tc.tile_pool(name="at_q", bufs=2) as at_q,
                    tc.tile_pool(name="at_e", bufs=4) as at_e,
                    tc.tile_pool(name="at_small", bufs=4) as at_small,
                    tc.tile_pool(name="at_ps", bufs=3, space="PSUM") as at_ps,
                    tc.tile_pool(name="at_ps_acc", bufs=2, space="PSUM") as at_ps_acc,
                    tc.tile_pool(name="at_ps_sum", bufs=2, space="PSUM") as at_ps_sum,
                ):
                    kT_sb = at_kv.tile([P, HKV, S], f32r)
                    v_sb = at_kv.tile([P, S // P, HKV * DK], f32r)
                    nc.sync.dma_start(kT_sb[:], kT_dram.rearrange("h p s -> p h s"))
                    nc.sync.dma_start(v_sb[:], v_dram.rearrange("t p v -> p t v"))

                    for s in range(4):
                        nb = NB_SCHED[s]
                        npair = nb // 2
                        qT_sl = at_q.tile([P, H, TILE_R], f32r, tag="qsl")
                        nc.sync.dma_start(
                            qT_sl[:],
                            qT_dram.rearrange("h p r -> p h r")[
                                :, :, s * TILE_R : (s + 1) * TILE_R
                            ],
                        )
                        for h in range(H):
                            kvh = h // GROUP
                            ps_at = at_ps_acc.tile([P, TILE_R], f32, tag="at")
                            ps_sum = at_ps_sum.tile([1, TILE_R], f32, tag="sum")
                            for pr in range(npair):
                                b0 = 2 * pr
                                ps_sc = at_ps.tile([P, 2 * TILE_R], f32, tag="sc")
                                for half in range(2):
                                    nc.tensor.matmul(
                                        ps_sc[:, half * TILE_R : (half + 1) * TILE_R],
                                        kT_sb[
                                            :,
                                            kvh,
                                            (b0 + half) * KB : (b0 + half + 1) * KB,
                                        ],
                                        qT_sl[:, h],
                                        start=True,
                                        stop=True,
                                    )
                                j = pr - (npair - 2)
                                if j >= 0:
                                    nc.vector.tensor_tensor(
                                        ps_sc[:], ps_sc[:], mask_sb[:, s, j], Add
                                    )
                                e_sb = at_e.tile([P, 2 * TILE_R], f32r, tag="e")
                                nc.scalar.activation(e_sb[:], ps_sc[:], Exp)
                                for half in range(2):
                                    b = b0 + half
                                    e_half = e_sb[
                                        :, half * TILE_R : (half + 1) * TILE_R
                                    ]
                                    nc.tensor.matmul(
                                        ps_at[:],
                                        v_sb[:, b, kvh * DK : (kvh + 1) * DK],
                                        e_half,
                                        start=(b == 0),
                                        stop=(b == nb - 1),
                                    )
                                    nc.tensor.matmul(
                                        ps_sum[:],
                                        ones_sb[:],
                                        e_half,
                                        start=(b == 0),
                                        stop=(b == nb - 1),
                                    )
                            rec_sb = at_small.tile([1, TILE_R], f32, tag="rec")
                            nc.vector.reciprocal(rec_sb[:], ps_sum[:])
                            bc_sb = at_small.tile([P, TILE_R], f32, tag="bc")
                            nc.gpsimd.partition_broadcast(bc_sb[:], rec_sb[:])
                            nc.vector.tensor_tensor(
                                attn_all[:, s, h], ps_at[:], bc_sb[:], Mult
                            )

                # ---- Phase 4: Wo ----
                with (
                    tc.tile_pool(name="wo_w", bufs=1) as wo_w,
                    tc.tile_pool(name="wo_out", bufs=3) as wo_out,
                    tc.tile_pool(name="wo_ps", bufs=3, space="PSUM") as wo_ps,
                ):
                    wo_sb = wo_w.tile([P, 16, D], bf16)
                    nc.sync.dma_start(wo_sb[:], wo_t)
                    for rc in range(8):
                        s, half = rc // 2, rc % 2
                        for oc in range(4):
                            ps_o = wo_ps.tile([P, 512], f32, tag="o")
                            for hc in range(16):
                                nc.tensor.matmul(
                                    ps_o[:],
                                    attn_all[:, s, hc, half * P : (half + 1) * P],
                                    wo_sb[:, hc, oc * 512 : (oc + 1) * 512],
                                    start=(hc == 0),
                                    stop=(hc == 15),
                                )
                            o_sb = wo_out.tile([P, 512], f32, tag="osb")
                            nc.any.tensor_copy(o_sb[:], ps_o[:])
                            nc.sync.dma_start(
                                out[
                                    rc * P : (rc + 1) * P, oc * 512 : (oc + 1) * 512
                                ],
                                o_sb[:],
                            )

    nc.finalize()
    return nc


def kernel(x, Wq1, Wq2, Wk1, Wk2, Wv1, Wv2, Wo):
    global LAST_RESULT
    from concourse.bass_utils import run_bass_kernel_spmd

    x = np.asarray(x, dtype=np.float32)
    Wq1 = np.asarray(Wq1, dtype=np.float32)
    Wq2 = np.asarray(Wq2, dtype=np.float32)
    Wk1 = np.asarray(Wk1, dtype=np.float32)
    Wk2 = np.asarray(Wk2, dtype=np.float32)
    Wv1 = np.asarray(Wv1, dtype=np.float32)
    Wv2 = np.asarray(Wv2, dtype=np.float32)
    Wo = np.asarray(Wo, dtype=np.float32)

    if "nc" not in _CACHE:
        _CACHE["nc"] = _build_nc()
    nc = _CACHE["nc"]

    wq2s = (Wq2 / np.sqrt(DK)).astype(np.float32)
    wo_bf = Wo.astype(ml_dtypes.bfloat16)
    masks = {p: _make_mask(p) for p in range(2)}
    rows = {p: _rows_sched(p) for p in range(2)}
    ones_np = np.ones((P, 1), np.float32)

    in_maps = []
    for core in range(8):
        batch, parity = core // 2, core % 2
        xb = x[batch]
        in_maps.append(
            {
                "xTkv": np.ascontiguousarray(xb.T),
                "xTq": np.ascontiguousarray(xb[rows[parity]].T),
                "wq1": Wq1,
                "wq2": wq2s,
                "wk1": Wk1,
                "wk2": Wk2,
                "wv1": Wv1,
                "wv2": Wv2,
                "wo": wo_bf,
                "maskin": masks[parity],
                "ones_in": ones_np,
            }
        )

    res = run_bass_kernel_spmd(nc, in_maps, core_ids=list(range(8)), trace=TRACE)
    LAST_RESULT = res

    out_full = np.empty((B, S, D), np.float32)
    for core in range(8):
        batch, parity = core // 2, core % 2
        out_full[batch][rows[parity]] = res.results[core]["out"]
    return out_full


# revision 6
# speedup vs baseline: 1.0062x; 1.0062x over previous
"""TRN2 Bass kernel for nn_CoreAttention_34875134444341.

Strategy (8 NeuronCores, no collectives):
  - Data-parallel over batch (4) x causal-balanced query-row split (2).
  - Each core: full K/V projections for its batch; Q projection for its
    1024 query rows (zig-zag tile assignment balances causal attention
    work); block attention in "transposed" orientation (keys on the
    partition axis) so no on-chip transposes are needed; final Wo matmul
    row-parallel (no reduction across cores).
  - All matmuls run as float32r (full-rate fp32 on the PE array) except
    the Wo stage which runs bf16.
  - Host gathers per-core outputs and inverse-permutes rows.
"""

import sys

sys.path.insert(0, "/opt/trn_rl_repo")

import numpy as np
import ml_dtypes

B, S, D = 4, 2048, 2048
H, HKV, DK = 16, 4, 128
RQ = RKV = 512
GROUP = H // HKV
P = 128

TILE_R = 256  # query rows per slot
KB = 128  # keys per block
NB_SCHED = [16, 12, 8, 4]  # key blocks per slot (same on every core)
TILE_ASSIGN = {0: [7, 5, 2, 0], 1: [6, 4, 3, 1]}  # slot -> query tile

ROWS_PER_CORE = 4 * TILE_R  # 1024

_CACHE = {}
TRACE = False
LAST_RESULT = None


def _rows_sched(parity):
    return np.concatenate(
        [np.arange(t * TILE_R, (t + 1) * TILE_R) for t in TILE_ASSIGN[parity]]
    )


def _make_mask(parity):
    """[128 key_in_block, 4 slots, 2 pairs, 512 (=2 blocks x 256 rows)] additive."""
    m = np.zeros((KB, 4, 2, 2 * TILE_R), np.float32)
    for s in range(4):
        t = TILE_ASSIGN[parity][s]
        nb = NB_SCHED[s]
        row_g = t * TILE_R + np.arange(TILE_R)
        for pr in range(2):  # last two pairs of the slot's schedule
            for half in range(2):
                blk = nb - 4 + 2 * pr + half
                key_g = blk * KB + np.arange(KB)
                bad = key_g[:, None] > row_g[None, :]
                sl = m[:, s, pr, half * TILE_R : (half + 1) * TILE_R]
                sl[bad] = -1e30
    return m


def _build_nc():
    import concourse.tile as tile
    from concourse import bacc, mybir

    f32 = mybir.dt.float32
    f32r = mybir.dt.float32r
    bf16 = mybir.dt.bfloat16
    Exp = mybir.ActivationFunctionType.Exp
    Mult = mybir.AluOpType.mult
    Add = mybir.AluOpType.add

    nc = bacc.Bacc("TRN2", target_bir_lowering=False, debug=False)

    xTkv = nc.dram_tensor("xTkv", [D, S], f32r, kind="ExternalInput")
    xTq = nc.dram_tensor("xTq", [D, ROWS_PER_CORE], f32r, kind="ExternalInput")
    wq1 = nc.dram_tensor("wq1", [D, RQ], f32r, kind="ExternalInput")
    wq2 = nc.dram_tensor("wq2", [RQ, H * DK], f32r, kind="ExternalInput")
    wk1 = nc.dram_tensor("wk1", [D, RKV], f32r, kind="ExternalInput")
    wk2 = nc.dram_tensor("wk2", [RKV, HKV * DK], f32r, kind="ExternalInput")
    wv1 = nc.dram_tensor("wv1", [D, RKV], f32r, kind="ExternalInput")
    wv2 = nc.dram_tensor("wv2", [RKV, HKV * DK], f32r, kind="ExternalInput")
    wo = nc.dram_tensor("wo", [D, D], bf16, kind="ExternalInput")
    maskin = nc.dram_tensor("maskin", [KB, 4, 2, 2 * TILE_R], f32, kind="ExternalInput")
    ones_in = nc.dram_tensor("ones_in", [P, 1], f32r, kind="ExternalInput")
    out = nc.dram_tensor("out", [ROWS_PER_CORE, D], f32, kind="ExternalOutput")

    # DRAM scratch (internal)
    kT_dram = nc.dram_tensor("kT_scratch", [HKV, P, S], f32r)
    v_dram = nc.dram_tensor("v_scratch", [S // P, P, HKV * DK], f32r)
    qT_dram = nc.dram_tensor("qT_scratch", [H, P, ROWS_PER_CORE], f32r)

    xTkv_t = xTkv.rearrange("(dc p) s -> p dc s", p=P)  # [128, 16, 2048]
    xTq_t = xTq.rearrange("(dc p) r -> p dc r", p=P)  # [128, 16, 1024]
    wq1_t = wq1.rearrange("(dc p) r -> p dc r", p=P)  # [128, 16, 512]
    wk1_t = wk1.rearrange("(dc p) r -> p dc r", p=P)
    wv1_t = wv1.rearrange("(dc p) r -> p dc r", p=P)
    wq2_t = wq2.rearrange("(rc p) h -> p rc h", p=P)  # [128, 4, 2048]
    wk2_t = wk2.rearrange("(rc p) h -> p rc h", p=P)  # [128, 4, 512]
    wv2_t = wv2.rearrange("(rc p) h -> p rc h", p=P)
    wo_t = wo.rearrange("(hc p) o -> p hc o", p=P)  # [128, 16, 2048]

    with tile.TileContext(nc) as tc:
        with tc.tile_pool(name="persist", bufs=1) as persist:
            ones_sb = persist.tile([P, 1], f32r)
            nc.sync.dma_start(ones_sb[:], ones_in[:])
            mask_sb = persist.tile([P, 4, 2, 2 * TILE_R], f32)
            nc.sync.dma_start(mask_sb[:], maskin[:])

            # ------- Phase 1: K then V projections -> DRAM scratch --------
            with (
                tc.tile_pool(name="kv_w1", bufs=1) as kv_w1,
                tc.tile_pool(name="kv_w2", bufs=1) as kv_w2,
                tc.tile_pool(name="kv_x", bufs=2) as kv_x,
                tc.tile_pool(name="kv_mid", bufs=2) as kv_mid,
                tc.tile_pool(name="kv_out", bufs=3) as kv_out,
                tc.tile_pool(name="kv_ps", bufs=4, space="PSUM") as kv_ps,
            ):
                for which in range(2):  # 0 = K, 1 = V
                    w1_t, w2_t = (wk1_t, wk2_t) if which == 0 else (wv1_t, wv2_t)
                    w1_sb = kv_w1.tile([P, 16, RKV], f32r, tag="w1")
                    w2_sb = kv_w2.tile([P, 4, HKV * DK], f32r, tag="w2")
                    nc.sync.dma_start(w1_sb[:], w1_t)
                    nc.sync.dma_start(w2_sb[:], w2_t)

                    for tcn in range(4):  # token 512-chunks
                        xt = kv_x.tile([P, 16, 512], f32r, tag="xt")
                        nc.sync.dma_start(
                            xt[:], xTkv_t[:, :, tcn * 512 : (tcn + 1) * 512]
                        )
                        mid = kv_mid.tile([P, 4, 512], f32r, tag="mid")
                        for rc in range(4):
                            ps_1 = kv_ps.tile([P, 512], f32, tag="ps1")
                            for dc in range(16):
                                nc.tensor.matmul(
                                    ps_1[:],
                                    w1_sb[:, dc, rc * P : (rc + 1) * P],
                                    xt[:, dc],
                                    start=(dc == 0),
                                    stop=(dc == 15),
                                )
                            nc.any.tensor_copy(mid[:, rc], ps_1[:])

                        if which == 0:
                            # kT [kvh*128, tokens]
                            for hc in range(HKV):
                                ps_2 = kv_ps.tile([P, 512], f32, tag="ps2")
                                for rc in range(4):
                                    nc.tensor.matmul(
                                        ps_2[:],
                                        w2_sb[:, rc, hc * P : (hc + 1) * P],
                                        mid[:, rc],
                                        start=(rc == 0),
                                        stop=(rc == 3),
                                    )
                                bounce = kv_out.tile([P, 512], f32r, tag="bo")
                                nc.any.tensor_copy(bounce[:], ps_2[:])
                                nc.sync.dma_start(
                                    kT_dram[hc, :, tcn * 512 : (tcn + 1) * 512],
                                    bounce[:],
                                )
                        else:
                            # v [tokens, kvhid]
                            for i in range(4):
                                ps_2 = kv_ps.tile([P, 512], f32, tag="ps2")
                                for rc in range(4):
                                    nc.tensor.matmul(
                                        ps_2[:],
                                        mid[:, rc, i * P : (i + 1) * P],
                                        w2_sb[:, rc],
                                        start=(rc == 0),
                                        stop=(rc == 3),
                                    )
                                bounce = kv_out.tile([P, 512], f32r, tag="bo")
                                nc.any.tensor_copy(bounce[:], ps_2[:])
                                nc.sync.dma_start(
                                    v_dram[tcn * 4 + i, :, :], bounce[:]
                                )

            # ------- Phase 2: Q projection -> DRAM scratch ----------------
            with (
                tc.tile_pool(name="q_w", bufs=1) as q_w,
                tc.tile_pool(name="q_x", bufs=2) as q_x,
                tc.tile_pool(name="q_mid", bufs=1) as q_mid,
                tc.tile_pool(name="q_out", bufs=3) as q_out,
                tc.tile_pool(name="q_ps", bufs=4, space="PSUM") as q_ps,
            ):
                wq1_sb = q_w.tile([P, 16, RQ], f32r)
                wq2_sb = q_w.tile([P, 4, H * DK], f32r)
                nc.sync.dma_start(wq1_sb[:], wq1_t)
                nc.sync.dma_start(wq2_sb[:], wq2_t)

                q1t = q_mid.tile([P, 4, ROWS_PER_CORE], f32r)
                for tcn in range(2):
                    xt = q_x.tile([P, 16, 512], f32r, tag="xtq")
                    nc.sync.dma_start(xt[:], xTq_t[:, :, tcn * 512 : (tcn + 1) * 512])
                    for rc in range(4):
                        ps_q = q_ps.tile([P, 512], f32, tag="psq1")
                        for dc in range(16):
                            nc.tensor.matmul(
                                ps_q[:],
                                wq1_sb[:, dc, rc * P : (rc + 1) * P],
                                xt[:, dc],
                                start=(dc == 0),
                                stop=(dc == 15),
                            )
                        nc.any.tensor_copy(
                            q1t[:, rc, tcn * 512 : (tcn + 1) * 512], ps_q[:]
                        )
                for h in range(H):
                    for tcn in range(2):
                        ps_qT = q_ps.tile([P, 512], f32, tag="psq2")
                        for rc in range(4):
                            nc.tensor.matmul(
                                ps_qT[:],
                                wq2_sb[:, rc, h * P : (h + 1) * P],
                                q1t[:, rc, tcn * 512 : (tcn + 1) * 512],
                                start=(rc == 0),
                                stop=(rc == 3),
                            )
                        qbounce = q_out.tile([P, 512], f32r, tag="qb")
                        nc.any.tensor_copy(qbounce[:], ps_qT[:])
                        nc.sync.dma_start(
                            qT_dram[h, :, tcn * 512 : (tcn + 1) * 512], qbounce[:]
                        )

            # ------- Phases 3+4 share attn_all ----------------------------
            with tc.tile_pool(name="attn_keep", bufs=1) as attn_keep:
                attn_all = attn_keep.tile([P, 4, H, TILE_R], bf16)

                # ---- Phase 3: attention ----
                with (
                    tc.tile_pool(name="at_kv", bufs=1) as at_kv,
                    tc.tile_pool(name="at_q", bufs=2) as at_q,
                    tc.tile_pool(name="at_e", bufs=4) as at_e,
                    tc.tile_pool(name="at_small", bufs=4) as at_small,
                    tc.tile_pool(name="at_ps", bufs=4, space="PSUM") as at_ps,
                    tc.tile_pool(name="at_ps_acc", bufs=2, space="PSUM") as at_ps_acc,
                    tc.tile_pool(name="at_ps_sum", bufs=2, space="PSUM") as at_ps_sum,
                ):
                    kT_sb = at_kv.tile([P, HKV, S], f32r)
                    v_sb = at_kv.tile([P, S // P, HKV * DK], f32r)
                    for qt in range(4):  # S quarters, earliest keys first
                        for kvh in range(HKV):
                            nc.sync.dma_start(
                                kT_sb[:, kvh, qt * 512 : (qt + 1) * 512],
                                kT_dram[kvh, :, qt * 512 : (qt + 1) * 512],
                            )
                        for i in range(4):
                            tt = qt * 4 + i
                            nc.sync.dma_start(v_sb[:, tt], v_dram[tt, :, :])

                    for s in (3, 2, 1, 0):
                        nb = NB_SCHED[s]
                        npair = nb // 2
                        qT_sl = at_q.tile([P, H, TILE_R], f32r, tag="qsl")
                        nc.sync.dma_start(
                            qT_sl[:],
                            qT_dram.rearrange("h p r -> p h r")[
                                :, :, s * TILE_R : (s + 1) * TILE_R
                            ],
                        )
                        for h in range(H):
                            kvh = h // GROUP
                            ps_at = at_ps_acc.tile([P, TILE_R], f32, tag="at")
                            ps_sum = at_ps_sum.tile([1, 2 * TILE_R], f32, tag="sum")
                            for pr in range(npair):
                                b0 = 2 * pr
                                ps_sc = at_ps.tile([P, 2 * TILE_R], f32, tag="sc")
                                for half in range(2):
                                    nc.tensor.matmul(
                                        ps_sc[:, half * TILE_R : (half + 1) * TILE_R],
                                        kT_sb[
                                            :,
                                            kvh,
                                            (b0 + half) * KB : (b0 + half + 1) * KB,
                                        ],
                                        qT_sl[:, h],
                                        start=True,
                                        stop=True,
                                    )
                                j = pr - (npair - 2)
                                if j >= 0:
                                    nc.vector.tensor_tensor(
                                        ps_sc[:], ps_sc[:], mask_sb[:, s, j], Add
                                    )
                                e_sb = at_e.tile([P, 2 * TILE_R], f32r, tag="e")
                                nc.scalar.activation(e_sb[:], ps_sc[:], Exp)
                                for half in range(2):
                                    b = b0 + half
                                    e_half = e_sb[
                                        :, half * TILE_R : (half + 1) * TILE_R
                                    ]
                                    nc.tensor.matmul(
                                        ps_at[:],
                                        v_sb[:, b, kvh * DK : (kvh + 1) * DK],
                                        e_half,
                                        start=(b == 0),
                                        stop=(b == nb - 1),
                                    )
                                nc.tensor.matmul(
                                    ps_sum[:],
                                    ones_sb[:],
                                    e_sb[:],
                                    start=(pr == 0),
                                    stop=(pr == npair - 1),
                                )
                            sums2_sb = at_small.tile([1, 2 * TILE_R], f32, tag="s2")
                            nc.vector.tensor_copy(sums2_sb[:], ps_sum[:])
                            sum_sb = at_small.tile([1, TILE_R], f32, tag="sumc")
                            nc.vector.tensor_tensor(
                                sum_sb[:],
                                sums2_sb[:, :TILE_R],
                                sums2_sb[:, TILE_R:],
                                Add,
                            )
                            rec_sb = at_small.tile([1, TILE_R], f32, tag="rec")
                            nc.vector.reciprocal(rec_sb[:], sum_sb[:])
                            bc_sb = at_small.tile([P, TILE_R], f32, tag="bc")
                            nc.gpsimd.partition_broadcast(bc_sb[:], rec_sb[:])
                            nc.vector.tensor_tensor(
                                attn_all[:, s, h], ps_at[:], bc_sb[:], Mult
                            )

                # ---- Phase 4: Wo ----
                with (
                    tc.tile_pool(name="wo_w", bufs=2) as wo_w,
                    tc.tile_pool(name="wo_out", bufs=3) as wo_out,
                    tc.tile_pool(name="wo_ps", bufs=3, space="PSUM") as wo_ps,
                ):
                    for oc in range(4):
                        wo_sb = wo_w.tile([P, 16, 512], bf16, tag="woc")
                        nc.sync.dma_start(
                            wo_sb[:], wo_t[:, :, oc * 512 : (oc + 1) * 512]
                        )
                        for rc in range(8):
                            s, half = rc // 2, rc % 2
                            ps_o = wo_ps.tile([P, 512], f32, tag="o")
                            for hc in range(16):
                                nc.tensor.matmul(
                                    ps_o[:],
                                    attn_all[:, s, hc, half * P : (half + 1) * P],
                                    wo_sb[:, hc],
                                    start=(hc == 0),
                                    stop=(hc == 15),
                                )
                            o_sb = wo_out.tile([P, 512], f32, tag="osb")
                            nc.any.tensor_copy(o_sb[:], ps_o[:])
                            nc.sync.dma_start(
                                out[
                                    rc * P : (rc + 1) * P, oc * 512 : (oc + 1) * 512
                                ],
                                o_sb[:],
                            )

    nc.finalize()
    return nc


def kernel(x, Wq1, Wq2, Wk1, Wk2, Wv1, Wv2, Wo):
    global LAST_RESULT
    from concourse.bass_utils import run_bass_kernel_spmd

    x = np.asarray(x, dtype=np.float32)
    Wq1 = np.asarray(Wq1, dtype=np.float32)
    Wq2 = np.asarray(Wq2, dtype=np.float32)
    Wk1 = np.asarray(Wk1, dtype=np.float32)
    Wk2 = np.asarray(Wk2, dtype=np.float32)
    Wv1 = np.asarray(Wv1, dtype=np.float32)
    Wv2 = np.asarray(Wv2, dtype=np.float32)
    Wo = np.asarray(Wo, dtype=np.float32)

    if "nc" not in _CACHE:
        _CACHE["nc"] = _build_nc()
    nc = _CACHE["nc"]

    wq2s = (Wq2 / np.sqrt(DK)).astype(np.float32)
    wo_bf = Wo.astype(ml_dtypes.bfloat16)
    masks = {p: _make_mask(p) for p in range(2)}
    rows = {p: _rows_sched(p) for p in range(2)}
    ones_np = np.ones((P, 1), np.float32)

    in_maps = []
    for core in range(8):
        batch, parity = core // 2, core % 2
        xb = x[batch]
        in_maps.append(
            {
                "xTkv": np.ascontiguousarray(xb.T),
                "xTq": np.ascontiguousarray(xb[rows[parity]].T),
                "wq1": Wq1,
                "wq2": wq2s,
                "wk1": Wk1,
                "wk2": Wk2,
                "wv1": Wv1,
                "wv2": Wv2,
                "wo": wo_bf,
                "maskin": masks[parity],
                "ones_in": ones_np,
            }
        )

    res = run_bass_kernel_spmd(nc, in_maps, core_ids=list(range(8)), trace=TRACE)
    LAST_RESULT = res

    out_full = np.empty((B, S, D), np.float32)
    for core in range(8):
        batch, parity = core // 2, core % 2
        out_full[batch][rows[parity]] = res.results[core]["out"]
    return out_full


# revision 7
# speedup vs baseline: 1.0592x; 1.0527x over previous
"""TRN2 Bass kernel for nn_CoreAttention_34875134444341.

Strategy (8 NeuronCores, no collectives):
  - Data-parallel over batch (4) x causal-balanced query-row split (2).
  - Each core: Q projection for its 1024 query rows (zig-zag tile
    assignment balances causal attention work) spilled to DRAM scratch;
    full K/V projections for its batch kept resident in SBUF; block
    attention in "transposed" orientation (keys on the partition axis)
    so no on-chip transposes are needed; final Wo matmul row-parallel
    (no reduction across cores).
  - All matmuls run as float32r (full-rate fp32 on the PE array) except
    the Wo stage which runs bf16.
  - Host gathers per-core outputs and inverse-permutes rows.
"""

import sys

sys.path.insert(0, "/opt/trn_rl_repo")

import numpy as np
import ml_dtypes

B, S, D = 4, 2048, 2048
H, HKV, DK = 16, 4, 128
RQ = RKV = 512
GROUP = H // HKV
P = 128

TILE_R = 256  # query rows per slot
KB = 128  # keys per block
NB_SCHED = [16, 12, 8, 4]  # key blocks per slot (same on every core)
TILE_ASSIGN = {0: [7, 5, 2, 0], 1: [6, 4, 3, 1]}  # slot -> query tile

ROWS_PER_CORE = 4 * TILE_R  # 1024

_CACHE = {}
TRACE = False
LAST_RESULT = None


def _rows_sched(parity):
    return np.concatenate(
        [np.arange(t * TILE_R, (t + 1) * TILE_R) for t in TILE_ASSIGN[parity]]
    )


def _make_mask(parity):
    """[128 key_in_block, 4 slots, 2 pairs, 512 (=2 blocks x 256 rows)] additive."""
    m = np.zeros((KB, 4, 2, 2 * TILE_R), np.float32)
    for s in range(4):
        t = TILE_ASSIGN[parity][s]
        nb = NB_SCHED[s]
        row_g = t * TILE_R + np.arange(TILE_R)
        for pr in range(2):  # last two pairs of the slot's schedule
            for half in range(2):
                blk = nb - 4 + 2 * pr + half
                key_g = blk * KB + np.arange(KB)
                bad = key_g[:, None] > row_g[None, :]
                sl = m[:, s, pr, half * TILE_R : (half + 1) * TILE_R]
                sl[bad] = -1e30
    return m


def _build_nc():
    import concourse.tile as tile
    from concourse import bacc, mybir

    f32 = mybir.dt.float32
    f32r = mybir.dt.float32r
    bf16 = mybir.dt.bfloat16
    Exp = mybir.ActivationFunctionType.Exp
    Mult = mybir.AluOpType.mult
    Add = mybir.AluOpType.add

    nc = bacc.Bacc("TRN2", target_bir_lowering=False, debug=False)

    xTkv = nc.dram_tensor("xTkv", [D, S], f32r, kind="ExternalInput")
    xTq = nc.dram_tensor("xTq", [D, ROWS_PER_CORE], f32r, kind="ExternalInput")
    wq1 = nc.dram_tensor("wq1", [D, RQ], f32r, kind="ExternalInput")
    wq2 = nc.dram_tensor("wq2", [RQ, H * DK], f32r, kind="ExternalInput")
    wk1 = nc.dram_tensor("wk1", [D, RKV], f32r, kind="ExternalInput")
    wk2 = nc.dram_tensor("wk2", [RKV, HKV * DK], f32r, kind="ExternalInput")
    wv1 = nc.dram_tensor("wv1", [D, RKV], f32r, kind="ExternalInput")
    wv2 = nc.dram_tensor("wv2", [RKV, HKV * DK], f32r, kind="ExternalInput")
    wo = nc.dram_tensor("wo", [D, D], bf16, kind="ExternalInput")
    maskin = nc.dram_tensor("maskin", [KB, 4, 2, 2 * TILE_R], f32, kind="ExternalInput")
    ones_in = nc.dram_tensor("ones_in", [P, 1], f32r, kind="ExternalInput")
    out = nc.dram_tensor("out", [ROWS_PER_CORE, D], f32, kind="ExternalOutput")

    qT_dram = nc.dram_tensor("qT_scratch", [H, P, ROWS_PER_CORE], f32r)

    xTkv_t = xTkv.rearrange("(dc p) s -> p dc s", p=P)  # [128, 16, 2048]
    xTq_t = xTq.rearrange("(dc p) r -> p dc r", p=P)  # [128, 16, 1024]
    wq1_t = wq1.rearrange("(dc p) r -> p dc r", p=P)  # [128, 16, 512]
    wk1_t = wk1.rearrange("(dc p) r -> p dc r", p=P)
    wv1_t = wv1.rearrange("(dc p) r -> p dc r", p=P)
    wq2_t = wq2.rearrange("(rc p) h -> p rc h", p=P)  # [128, 4, 2048]
    wk2_t = wk2.rearrange("(rc p) h -> p rc h", p=P)  # [128, 4, 512]
    wv2_t = wv2.rearrange("(rc p) h -> p rc h", p=P)
    wo_t = wo.rearrange("(hc p) o -> p hc o", p=P)  # [128, 16, 2048]

    with tile.TileContext(nc) as tc:
        with tc.tile_pool(name="persist", bufs=1) as persist:
            ones_sb = persist.tile([P, 1], f32r)
            nc.sync.dma_start(ones_sb[:], ones_in[:])

            # ------- Phase 1: Q projection -> DRAM scratch ----------------
            with (
                tc.tile_pool(name="q_w", bufs=1) as q_w,
                tc.tile_pool(name="q_x", bufs=2) as q_x,
                tc.tile_pool(name="q_mid", bufs=1) as q_mid,
                tc.tile_pool(name="q_out", bufs=3) as q_out,
                tc.tile_pool(name="q_ps", bufs=4, space="PSUM") as q_ps,
            ):
                wq1_sb = q_w.tile([P, 16, RQ], f32r)
                nc.sync.dma_start(wq1_sb[:], wq1_t)
                xts = []
                for tcn in range(2):
                    xt = q_x.tile([P, 16, 512], f32r, tag="xtq")
                    nc.sync.dma_start(xt[:], xTq_t[:, :, tcn * 512 : (tcn + 1) * 512])
                    xts.append(xt)
                wq2_sb = q_w.tile([P, 4, H * DK], f32r)
                nc.sync.dma_start(wq2_sb[:], wq2_t)

                q1t = q_mid.tile([P, 4, ROWS_PER_CORE], f32r)
                for tcn in range(2):
                    xt = xts[tcn]
                    for rc in range(4):
                        ps_q = q_ps.tile([P, 512], f32, tag="psq1")
                        for dc in range(16):
                            nc.tensor.matmul(
                                ps_q[:],
                                wq1_sb[:, dc, rc * P : (rc + 1) * P],
                                xt[:, dc],
                                start=(dc == 0),
                                stop=(dc == 15),
                            )
                        nc.any.tensor_copy(
                            q1t[:, rc, tcn * 512 : (tcn + 1) * 512], ps_q[:]
                        )
                for h in range(H):
                    for tcn in range(2):
                        ps_qT = q_ps.tile([P, 512], f32, tag="psq2")
                        for rc in range(4):
                            nc.tensor.matmul(
                                ps_qT[:],
                                wq2_sb[:, rc, h * P : (h + 1) * P],
                                q1t[:, rc, tcn * 512 : (tcn + 1) * 512],
                                start=(rc == 0),
                                stop=(rc == 3),
                            )
                        qbounce = q_out.tile([P, 512], f32r, tag="qb")
                        nc.any.tensor_copy(qbounce[:], ps_qT[:])
                        nc.sync.dma_start(
                            qT_dram[h, :, tcn * 512 : (tcn + 1) * 512], qbounce[:]
                        )

            # ------- kT/v stay resident from here on ----------------------
            with tc.tile_pool(name="kv_keep", bufs=1) as kv_keep:
                kT_sb = kv_keep.tile([P, HKV, S], f32r)
                v_sb = kv_keep.tile([P, S // P, HKV * DK], f32r)

                # ---- Phase 2: K then V projections (resident outputs) ----
                with (
                    tc.tile_pool(name="kv_w1", bufs=1) as kv_w1,
                    tc.tile_pool(name="kv_w2", bufs=1) as kv_w2,
                    tc.tile_pool(name="kv_x", bufs=2) as kv_x,
                    tc.tile_pool(name="kv_mid", bufs=1) as kv_mid,
                    tc.tile_pool(name="kv_ps", bufs=4, space="PSUM") as kv_ps,
                ):
                    for which in range(2):  # 0 = K, 1 = V
                        w1_t, w2_t = (wk1_t, wk2_t) if which == 0 else (wv1_t, wv2_t)
                        w1_sb = kv_w1.tile([P, 16, RKV], f32r, tag="w1")
                        nc.sync.dma_start(w1_sb[:], w1_t)
                        w2_sb = kv_w2.tile([P, 4, HKV * DK], f32r, tag="w2")
                        nc.sync.dma_start(w2_sb[:], w2_t)

                        for tcn in range(4):  # token 512-chunks
                            xt = kv_x.tile([P, 16, 512], f32r, tag="xt")
                            nc.sync.dma_start(
                                xt[:], xTkv_t[:, :, tcn * 512 : (tcn + 1) * 512]
                            )
                            mid = kv_mid.tile([P, 4, 512], f32r, tag="mid")
                            for rc in range(4):
                                ps_1 = kv_ps.tile([P, 512], f32, tag="ps1")
                                for dc in range(16):
                                    nc.tensor.matmul(
                                        ps_1[:],
                                        w1_sb[:, dc, rc * P : (rc + 1) * P],
                                        xt[:, dc],
                                        start=(dc == 0),
                                        stop=(dc == 15),
                                    )
                                nc.any.tensor_copy(mid[:, rc], ps_1[:])

                            if which == 0:
                                for hc in range(HKV):
                                    ps_2 = kv_ps.tile([P, 512], f32, tag="ps2")
                                    for rc in range(4):
                                        nc.tensor.matmul(
                                            ps_2[:],
                                            w2_sb[:, rc, hc * P : (hc + 1) * P],
                                            mid[:, rc],
                                            start=(rc == 0),
                                            stop=(rc == 3),
                                        )
                                    nc.any.tensor_copy(
                                        kT_sb[:, hc, tcn * 512 : (tcn + 1) * 512],
                                        ps_2[:],
                                    )
                            else:
                                for i in range(4):
                                    ps_2 = kv_ps.tile([P, 512], f32, tag="ps2")
                                    for rc in range(4):
                                        nc.tensor.matmul(
                                            ps_2[:],
                                            mid[:, rc, i * P : (i + 1) * P],
                                            w2_sb[:, rc],
                                            start=(rc == 0),
                                            stop=(rc == 3),
                                        )
                                    nc.any.tensor_copy(v_sb[:, tcn * 4 + i], ps_2[:])

                # ---- Phases 3+4 share attn_all ----
                with tc.tile_pool(name="attn_keep", bufs=1) as attn_keep:
                    attn_all = attn_keep.tile([P, 4, H, TILE_R], bf16)

                    with (
                        tc.tile_pool(name="at_m", bufs=1) as at_m,
                        tc.tile_pool(name="at_q", bufs=2) as at_q,
                        tc.tile_pool(name="at_e", bufs=4) as at_e,
                        tc.tile_pool(name="at_small", bufs=4) as at_small,
                        tc.tile_pool(name="at_ps", bufs=4, space="PSUM") as at_ps,
                        tc.tile_pool(name="at_ps_acc", bufs=2, space="PSUM") as at_ps_acc,
                        tc.tile_pool(name="at_ps_sum", bufs=2, space="PSUM") as at_ps_sum,
                    ):
                        mask_sb = at_m.tile([P, 4, 2, 2 * TILE_R], f32)
                        nc.sync.dma_start(mask_sb[:], maskin[:])

                        for s in (3, 2, 1, 0):
                            nb = NB_SCHED[s]
                            npair = nb // 2
                            qT_sl = at_q.tile([P, H, TILE_R], f32r, tag="qsl")
                            nc.sync.dma_start(
                                qT_sl[:],
                                qT_dram.rearrange("h p r -> p h r")[
                                    :, :, s * TILE_R : (s + 1) * TILE_R
                                ],
                            )
                            for h in range(H):
                                kvh = h // GROUP
                                ps_at = at_ps_acc.tile([P, TILE_R], f32, tag="at")
                                ps_sum = at_ps_sum.tile(
                                    [1, 2 * TILE_R], f32, tag="sum"
                                )
                                for pr in range(npair):
                                    b0 = 2 * pr
                                    ps_sc = at_ps.tile([P, 2 * TILE_R], f32, tag="sc")
                                    for half in range(2):
                                        nc.tensor.matmul(
                                            ps_sc[
                                                :, half * TILE_R : (half + 1) * TILE_R
                                            ],
                                            kT_sb[
                                                :,
                                                kvh,
                                                (b0 + half) * KB : (b0 + half + 1)
                                                * KB,
                                            ],
                                            qT_sl[:, h],
                                            start=True,
                                            stop=True,
                                        )
                                    j = pr - (npair - 2)
                                    if j >= 0:
                                        nc.vector.tensor_tensor(
                                            ps_sc[:], ps_sc[:], mask_sb[:, s, j], Add
                                        )
                                    e_sb = at_e.tile([P, 2 * TILE_R], f32r, tag="e")
                                    nc.scalar.activation(e_sb[:], ps_sc[:], Exp)
                                    for half in range(2):
                                        b = b0 + half
                                        e_half = e_sb[
                                            :, half * TILE_R : (half + 1) * TILE_R
                                        ]
                                        nc.tensor.matmul(
                                            ps_at[:],
                                            v_sb[:, b, kvh * DK : (kvh + 1) * DK],
                                            e_half,
                                            start=(b == 0),
                                            stop=(b == nb - 1),
                                        )
                                    nc.tensor.matmul(
                                        ps_sum[:],
                                        ones_sb[:],
                                        e_sb[:],
                                        start=(pr == 0),
                                        stop=(pr == npair - 1),
                                    )
                                sums2_sb = at_small.tile(
                                    [1, 2 * TILE_R], f32, tag="s2"
                                )
                                nc.vector.tensor_copy(sums2_sb[:], ps_sum[:])
                                sum_sb = at_small.tile([1, TILE_R], f32, tag="sumc")
                                nc.vector.tensor_tensor(
                                    sum_sb[:],
                                    sums2_sb[:, :TILE_R],
                                    sums2_sb[:, TILE_R:],
                                    Add,
                                )
                                rec_sb = at_small.tile([1, TILE_R], f32, tag="rec")
                                nc.vector.reciprocal(rec_sb[:], sum_sb[:])
                                bc_sb = at_small.tile([P, TILE_R], f32, tag="bc")
                                nc.gpsimd.partition_broadcast(bc_sb[:], rec_sb[:])
                                nc.vector.tensor_tensor(
                                    attn_all[:, s, h], ps_at[:], bc_sb[:], Mult
                                )

                    # ---- Phase 4: Wo ----
                    with (
                        tc.tile_pool(name="wo_w", bufs=2) as wo_w,
                        tc.tile_pool(name="wo_out", bufs=3) as wo_out,
                        tc.tile_pool(name="wo_ps", bufs=3, space="PSUM") as wo_ps,
                    ):
                        for oc in range(4):
                            wo_sb = wo_w.tile([P, 16, 512], bf16, tag="woc")
                            nc.sync.dma_start(
                                wo_sb[:], wo_t[:, :, oc * 512 : (oc + 1) * 512]
                            )
                            for rc in range(8):
                                s, half = rc // 2, rc % 2
                                ps_o = wo_ps.tile([P, 512], f32, tag="o")
                                for hc in range(16):
                                    nc.tensor.matmul(
                                        ps_o[:],
                                        attn_all[
                                            :, s, hc, half * P : (half + 1) * P
                                        ],
                                        wo_sb[:, hc],
                                        start=(hc == 0),
                                        stop=(hc == 15),
                                    )
                                o_sb = wo_out.tile([P, 512], f32, tag="osb")
                                nc.vector.tensor_copy(o_sb[:], ps_o[:])
                                nc.sync.dma_start(
                                    out[
                                        rc * P : (rc + 1) * P,
                                        oc * 512 : (oc + 1) * 512,
                                    ],
                                    o_sb[:],
                                )

    nc.finalize()
    return nc


def kernel(x, Wq1, Wq2, Wk1, Wk2, Wv1, Wv2, Wo):
    global LAST_RESULT
    from concourse.bass_utils import run_bass_kernel_spmd

    x = np.asarray(x, dtype=np.float32)
    Wq1 = np.asarray(Wq1, dtype=np.float32)
    Wq2 = np.asarray(Wq2, dtype=np.float32)
    Wk1 = np.asarray(Wk1, dtype=np.float32)
    Wk2 = np.asarray(Wk2, dtype=np.float32)
    Wv1 = np.asarray(Wv1, dtype=np.float32)
    Wv2 = np.asarray(Wv2, dtype=np.float32)
    Wo = np.asarray(Wo, dtype=np.float32)

    if "nc" not in _CACHE:
        _CACHE["nc"] = _build_nc()
    nc = _CACHE["nc"]

    wq2s = (Wq2 / np.sqrt(DK)).astype(np.float32)
    wo_bf = Wo.astype(ml_dtypes.bfloat16)
    masks = {p: _make_mask(p) for p in range(2)}
    rows = {p: _rows_sched(p) for p in range(2)}
    ones_np = np.ones((P, 1), np.float32)

    in_maps = []
    for core in range(8):
        batch, parity = core // 2, core % 2
        xb = x[batch]
        in_maps.append(
            {
                "xTkv": np.ascontiguousarray(xb.T),
                "xTq": np.ascontiguousarray(xb[rows[parity]].T),
                "wq1": Wq1,
                "wq2": wq2s,
                "wk1": Wk1,
                "wk2": Wk2,
                "wv1": Wv1,
                "wv2": Wv2,
                "wo": wo_bf,
                "maskin": masks[parity],
                "ones_in": ones_np,
            }
        )

    res = run_bass_kernel_spmd(nc, in_maps, core_ids=list(range(8)), trace=TRACE)
    LAST_RESULT = res

    out_full = np.empty((B, S, D), np.float32)
    for core in range(8):
        batch, parity = core // 2, core % 2
        out_full[batch][rows[parity]] = res.results[core]["out"]
    return out_full


# revision 9
# speedup vs baseline: 1.0792x; 1.0188x over previous
"""TRN2 Bass kernel for nn_CoreAttention_34875134444341.

Strategy (8 NeuronCores, no collectives):
  - Data-parallel over batch (4) x causal-balanced query-row split (2).
  - Each core: Q projection for its 1024 query rows (zig-zag tile
    assignment balances causal attention work) spilled to DRAM scratch;
    full K/V projections for its batch kept resident in SBUF; block
    attention in "transposed" orientation (keys on the partition axis)
    so no on-chip transposes are needed; final Wo matmul row-parallel
    (no reduction across cores).
  - All matmuls run as float32r (full-rate fp32 on the PE array) except
    the Wo stage which runs bf16.
  - Host gathers per-core outputs and inverse-permutes rows.
"""

import sys

sys.path.insert(0, "/opt/trn_rl_repo")

import numpy as np
import ml_dtypes

B, S, D = 4, 2048, 2048
H, HKV, DK = 16, 4, 128
RQ = RKV = 512
GROUP = H // HKV
P = 128

TILE_R = 256  # query rows per slot
KB = 128  # keys per block
NB_SCHED = [16, 12, 8, 4]  # key blocks per slot (same on every core)
TILE_ASSIGN = {0: [7, 5, 2, 0], 1: [6, 4, 3, 1]}  # slot -> query tile

ROWS_PER_CORE = 4 * TILE_R  # 1024

_CACHE = {}
TRACE = False
LAST_RESULT = None


def _rows_sched(parity):
    return np.concatenate(
        [np.arange(t * TILE_R, (t + 1) * TILE_R) for t in TILE_ASSIGN[parity]]
    )


def _make_mask(parity):
    """[128 key_in_block, 4 slots, 2 pairs, 512 (=2 blocks x 256 rows)] additive."""
    m = np.zeros((KB, 4, 2, 2 * TILE_R), np.float32)
    for s in range(4):
        t = TILE_ASSIGN[parity][s]
        nb = NB_SCHED[s]
        row_g = t * TILE_R + np.arange(TILE_R)
        for pr in range(2):  # last two pairs of the slot's schedule
            for half in range(2):
                blk = nb - 4 + 2 * pr + half
                key_g = blk * KB + np.arange(KB)
                bad = key_g[:, None] > row_g[None, :]
                sl = m[:, s, pr, half * TILE_R : (half + 1) * TILE_R]
                sl[bad] = -1e30
    return m


def _build_nc():
    import concourse.tile as tile
    from concourse import bacc, mybir

    f32 = mybir.dt.float32
    f32r = mybir.dt.float32r
    bf16 = mybir.dt.bfloat16
    Exp = mybir.ActivationFunctionType.Exp
    Mult = mybir.AluOpType.mult
    Add = mybir.AluOpType.add

    nc = bacc.Bacc("TRN2", target_bir_lowering=False, debug=False)

    xTkv = nc.dram_tensor("xTkv", [D, S], f32r, kind="ExternalInput")
    xTq = nc.dram_tensor("xTq", [D, ROWS_PER_CORE], f32r, kind="ExternalInput")
    wq1 = nc.dram_tensor("wq1", [D, RQ], f32r, kind="ExternalInput")
    wq2 = nc.dram_tensor("wq2", [RQ, H * DK], f32r, kind="ExternalInput")
    wk1 = nc.dram_tensor("wk1", [D, RKV], f32r, kind="ExternalInput")
    wk2 = nc.dram_tensor("wk2", [RKV, HKV * DK], f32r, kind="ExternalInput")
    wv1 = nc.dram_tensor("wv1", [D, RKV], f32r, kind="ExternalInput")
    wv2 = nc.dram_tensor("wv2", [RKV, HKV * DK], f32r, kind="ExternalInput")
    wo = nc.dram_tensor("wo", [D, D], bf16, kind="ExternalInput")
    maskin = nc.dram_tensor("maskin", [KB, 4, 2, 2 * TILE_R], f32, kind="ExternalInput")
    ones_in = nc.dram_tensor("ones_in", [P, 1], f32r, kind="ExternalInput")
    out = nc.dram_tensor("out", [ROWS_PER_CORE, D], f32, kind="ExternalOutput")

    qT_dram = nc.dram_tensor("qT_scratch", [H, P, ROWS_PER_CORE], f32r)

    xTkv_t = xTkv.rearrange("(dc p) s -> p dc s", p=P)  # [128, 16, 2048]
    xTq_t = xTq.rearrange("(dc p) r -> p dc r", p=P)  # [128, 16, 1024]
    wq1_t = wq1.rearrange("(dc p) r -> p dc r", p=P)  # [128, 16, 512]
    wk1_t = wk1.rearrange("(dc p) r -> p dc r", p=P)
    wv1_t = wv1.rearrange("(dc p) r -> p dc r", p=P)
    wq2_t = wq2.rearrange("(rc p) h -> p rc h", p=P)  # [128, 4, 2048]
    wk2_t = wk2.rearrange("(rc p) h -> p rc h", p=P)  # [128, 4, 512]
    wv2_t = wv2.rearrange("(rc p) h -> p rc h", p=P)
    wo_t = wo.rearrange("(hc p) o -> p hc o", p=P)  # [128, 16, 2048]

    with tile.TileContext(nc) as tc:
        with tc.tile_pool(name="persist", bufs=1) as persist:
            ones_sb = persist.tile([P, 1], f32r)
            nc.sync.dma_start(ones_sb[:], ones_in[:])

            # ------- Phase 1: Q projection -> DRAM scratch ----------------
            with (
                tc.tile_pool(name="q_w", bufs=1) as q_w,
                tc.tile_pool(name="q_x", bufs=2) as q_x,
                tc.tile_pool(name="q_mid", bufs=1) as q_mid,
                tc.tile_pool(name="q_out", bufs=3) as q_out,
                tc.tile_pool(name="q_ps", bufs=4, space="PSUM") as q_ps,
            ):
                wq1_sb = q_w.tile([P, 16, RQ], f32r)
                nc.sync.dma_start(wq1_sb[:], wq1_t)
                xts = []
                for tcn in range(2):
                    xt = q_x.tile([P, 16, 512], f32r, tag="xtq")
                    nc.sync.dma_start(xt[:], xTq_t[:, :, tcn * 512 : (tcn + 1) * 512])
                    xts.append(xt)
                wq2_sb = q_w.tile([P, 4, H * DK], f32r)
                nc.sync.dma_start(wq2_sb[:], wq2_t)

                q1t = q_mid.tile([P, 4, ROWS_PER_CORE], f32r)
                for tcn in range(2):
                    xt = xts[tcn]
                    for rc in range(4):
                        ps_q = q_ps.tile([P, 512], f32, tag="psq1")
                        for dc in range(16):
                            nc.tensor.matmul(
                                ps_q[:],
                                wq1_sb[:, dc, rc * P : (rc + 1) * P],
                                xt[:, dc],
                                start=(dc == 0),
                                stop=(dc == 15),
                            )
                        nc.any.tensor_copy(
                            q1t[:, rc, tcn * 512 : (tcn + 1) * 512], ps_q[:]
                        )
                for h in range(H):
                    for tcn in range(2):
                        ps_qT = q_ps.tile([P, 512], f32, tag="psq2")
                        for rc in range(4):
                            nc.tensor.matmul(
                                ps_qT[:],
                                wq2_sb[:, rc, h * P : (h + 1) * P],
                                q1t[:, rc, tcn * 512 : (tcn + 1) * 512],
                                start=(rc == 0),
                                stop=(rc == 3),
                            )
                        qbounce = q_out.tile([P, 512], f32r, tag="qb")
                        nc.any.tensor_copy(qbounce[:], ps_qT[:])
                        nc.sync.dma_start(
                            qT_dram[h, :, tcn * 512 : (tcn + 1) * 512], qbounce[:]
                        )

            # ------- kT/v stay resident from here on ----------------------
            with tc.tile_pool(name="kv_keep", bufs=1) as kv_keep:
                kT_sb = kv_keep.tile([P, HKV, S], f32r)
                v_sb = kv_keep.tile([P, S // P, HKV * DK], f32r)

                # ---- Phase 2: K then V projections (resident outputs) ----
                with (
                    tc.tile_pool(name="kv_w1", bufs=1) as kv_w1,
                    tc.tile_pool(name="kv_w2", bufs=1) as kv_w2,
                    tc.tile_pool(name="kv_x", bufs=2) as kv_x,
                    tc.tile_pool(name="kv_mid", bufs=1) as kv_mid,
                    tc.tile_pool(name="kv_ps", bufs=4, space="PSUM") as kv_ps,
                ):
                    for which in range(2):  # 0 = K, 1 = V
                        w1_t, w2_t = (wk1_t, wk2_t) if which == 0 else (wv1_t, wv2_t)
                        w1_sb = kv_w1.tile([P, 16, RKV], f32r, tag="w1")
                        for dq in range(4):
                            nc.sync.dma_start(
                                w1_sb[:, dq * 4 : (dq + 1) * 4], w1_t[:, dq * 4 : (dq + 1) * 4]
                            )
                        w2_sb = kv_w2.tile([P, 4, HKV * DK], f32r, tag="w2")
                        nc.sync.dma_start(w2_sb[:], w2_t)

                        for tcn in range(4):  # token 512-chunks
                            xt = kv_x.tile([P, 16, 512], f32r, tag="xt")
                            nc.sync.dma_start(
                                xt[:], xTkv_t[:, :, tcn * 512 : (tcn + 1) * 512]
                            )
                            mid = kv_mid.tile([P, 4, 512], f32r, tag="mid")
                            for rc in range(4):
                                ps_1 = kv_ps.tile([P, 512], f32, tag="ps1")
                                for dc in range(16):
                                    nc.tensor.matmul(
                                        ps_1[:],
                                        w1_sb[:, dc, rc * P : (rc + 1) * P],
                                        xt[:, dc],
                                        start=(dc == 0),
                                        stop=(dc == 15),
                                    )
                                nc.any.tensor_copy(mid[:, rc], ps_1[:])

                            if which == 0:
                                for hc in range(HKV):
                                    ps_2 = kv_ps.tile([P, 512], f32, tag="ps2")
                                    for rc in range(4):
                                        nc.tensor.matmul(
                                            ps_2[:],
                                            w2_sb[:, rc, hc * P : (hc + 1) * P],
                                            mid[:, rc],
                                            start=(rc == 0),
                                            stop=(rc == 3),
                                        )
                                    nc.any.tensor_copy(
                                        kT_sb[:, hc, tcn * 512 : (tcn + 1) * 512],
                                        ps_2[:],
                                    )
                            else:
                                for i in range(4):
                                    ps_2 = kv_ps.tile([P, 512], f32, tag="ps2")
                                    for rc in range(4):
                                        nc.tensor.matmul(
                                            ps_2[:],
                                            mid[:, rc, i * P : (i + 1) * P],
                                            w2_sb[:, rc],
                                            start=(rc == 0),
                                            stop=(rc == 3),
                                        )
                                    nc.any.tensor_copy(v_sb[:, tcn * 4 + i], ps_2[:])

                # ---- Phases 3+4 share attn_all ----
                with tc.tile_pool(name="attn_keep", bufs=1) as attn_keep:
                    attn_all = attn_keep.tile([P, 4, H, TILE_R], bf16)

                    with (
                        tc.tile_pool(name="at_m", bufs=1) as at_m,
                        tc.tile_pool(name="at_q", bufs=2) as at_q,
                        tc.tile_pool(name="at_e", bufs=4) as at_e,
                        tc.tile_pool(name="at_small", bufs=4) as at_small,
                        tc.tile_pool(name="at_ps", bufs=3, space="PSUM") as at_ps,
                        tc.tile_pool(name="at_ps_acc", bufs=3, space="PSUM") as at_ps_acc,
                        tc.tile_pool(name="at_ps_sum", bufs=2, space="PSUM") as at_ps_sum,
                    ):
                        mask_sb = at_m.tile([P, 4, 2, 2 * TILE_R], f32)
                        nc.sync.dma_start(mask_sb[:], maskin[:])

                        for s in (3, 2, 1, 0):
                            nb = NB_SCHED[s]
                            npair = nb // 2
                            qT_sl = at_q.tile([P, H, TILE_R], f32r, tag="qsl")
                            nc.sync.dma_start(
                                qT_sl[:],
                                qT_dram.rearrange("h p r -> p h r")[
                                    :, :, s * TILE_R : (s + 1) * TILE_R
                                ],
                            )
                            for h in range(H):
                                kvh = h // GROUP
                                ps_at = at_ps_acc.tile([P, TILE_R], f32, tag="at")
                                ps_sum = at_ps_sum.tile([1, TILE_R], f32, tag="sum")
                                for pr in range(npair):
                                    b0 = 2 * pr
                                    ps_sc = at_ps.tile([P, 2 * TILE_R], f32, tag="sc")
                                    for half in range(2):
                                        nc.tensor.matmul(
                                            ps_sc[
                                                :, half * TILE_R : (half + 1) * TILE_R
                                            ],
                                            kT_sb[
                                                :,
                                                kvh,
                                                (b0 + half) * KB : (b0 + half + 1)
                                                * KB,
                                            ],
                                            qT_sl[:, h],
                                            start=True,
                                            stop=True,
                                        )
                                    j = pr - (npair - 2)
                                    if j >= 0:
                                        nc.vector.tensor_tensor(
                                            ps_sc[:], ps_sc[:], mask_sb[:, s, j], Add
                                        )
                                    e_sb = at_e.tile([P, 2 * TILE_R], f32r, tag="e")
                                    nc.scalar.activation(e_sb[:], ps_sc[:], Exp)
                                    for half in range(2):
                                        b = b0 + half
                                        e_half = e_sb[
                                            :, half * TILE_R : (half + 1) * TILE_R
                                        ]
                                        nc.tensor.matmul(
                                            ps_at[:],
                                            v_sb[:, b, kvh * DK : (kvh + 1) * DK],
                                            e_half,
                                            start=(b == 0),
                                            stop=(b == nb - 1),
                                        )
                                        nc.tensor.matmul(
                                            ps_sum[:],
                                            ones_sb[:],
                                            e_half,
                                            start=(b == 0),
                                            stop=(b == nb - 1),
                                        )
                                rec_sb = at_small.tile([1, TILE_R], f32, tag="rec")
                                nc.vector.reciprocal(rec_sb[:], ps_sum[:])
                                bc_sb = at_small.tile([P, TILE_R], f32, tag="bc")
                                nc.gpsimd.partition_broadcast(bc_sb[:], rec_sb[:])
                                nc.vector.tensor_tensor(
                                    attn_all[:, s, h], ps_at[:], bc_sb[:], Mult
                                )

                    # ---- Phase 4: Wo ----
                    with (
                        tc.tile_pool(name="wo_w", bufs=2) as wo_w,
                        tc.tile_pool(name="wo_out", bufs=3) as wo_out,
                        tc.tile_pool(name="wo_ps", bufs=3, space="PSUM") as wo_ps,
                    ):
                        for oc in range(4):
                            wo_sb = wo_w.tile([P, 16, 512], bf16, tag="woc")
                            nc.sync.dma_start(
                                wo_sb[:], wo_t[:, :, oc * 512 : (oc + 1) * 512]
                            )
                            for rc in range(8):
                                s, half = rc // 2, rc % 2
                                ps_o = wo_ps.tile([P, 512], f32, tag="o")
                                for hc in range(16):
                                    nc.tensor.matmul(
                                        ps_o[:],
                                        attn_all[
                                            :, s, hc, half * P : (half + 1) * P
                                        ],
                                        wo_sb[:, hc],
                                        start=(hc == 0),
                                        stop=(hc == 15),
                                    )
                                o_sb = wo_out.tile([P, 512], f32, tag="osb")
                                nc.vector.tensor_copy(o_sb[:], ps_o[:])
                                nc.sync.dma_start(
                                    out[
                                        rc * P : (rc + 1) * P,
                                        oc * 512 : (oc + 1) * 512,
                                    ],
                                    o_sb[:],
                                )

    nc.finalize()
    return nc


def kernel(x, Wq1, Wq2, Wk1, Wk2, Wv1, Wv2, Wo):
    global LAST_RESULT
    from concourse.bass_utils import run_bass_kernel_spmd

    x = np.asarray(x, dtype=np.float32)
    Wq1 = np.asarray(Wq1, dtype=np.float32)
    Wq2 = np.asarray(Wq2, dtype=np.float32)
    Wk1 = np.asarray(Wk1, dtype=np.float32)
    Wk2 = np.asarray(Wk2, dtype=np.float32)
    Wv1 = np.asarray(Wv1, dtype=np.float32)
    Wv2 = np.asarray(Wv2, dtype=np.float32)
    Wo = np.asarray(Wo, dtype=np.float32)

    if "nc" not in _CACHE:
        _CACHE["nc"] = _build_nc()
    nc = _CACHE["nc"]

    wq2s = (Wq2 / np.sqrt(DK)).astype(np.float32)
    wo_bf = Wo.astype(ml_dtypes.bfloat16)
    masks = {p: _make_mask(p) for p in range(2)}
    rows = {p: _rows_sched(p) for p in range(2)}
    ones_np = np.ones((P, 1), np.float32)

    in_maps = []
    for core in range(8):
        batch, parity = core // 2, core % 2
        xb = x[batch]
        in_maps.append(
            {
                "xTkv": np.ascontiguousarray(xb.T),
                "xTq": np.ascontiguousarray(xb[rows[parity]].T),
                "wq1": Wq1,
                "wq2": wq2s,
                "wk1": Wk1,
                "wk2": Wk2,
                "wv1": Wv1,
                "wv2": Wv2,
                "wo": wo_bf,
                "maskin": masks[parity],
                "ones_in": ones_np,
            }
        )

    res = run_bass_kernel_spmd(nc, in_maps, core_ids=list(range(8)), trace=TRACE)
    LAST_RESULT = res

    out_full = np.empty((B, S, D), np.float32)
    for core in range(8):
        batch, parity = core // 2, core % 2
        out_full[batch][rows[parity]] = res.results[core]["out"]
    return out_full
